# revision 3
# baseline (speedup 1.0000x reference)
"""EnhancedFlowGNN forward pass on 8 Trainium2 NeuronCores (Bass/Tile).

Strategy (edge parallelism aligned with a node partition, no all-reduce):
  - Host sorts edges by destination ("row") and shards them by row range so
    core i owns nodes [i*6250, (i+1)*6250) and every edge targeting them.
  - segment_sum scatter = one-hot matmul into PSUM per 128-node block:
    U_w[e, n] = (row_rel[e] == n) * ex[e];  acc += U_w.T @ gathered_rows.
    Softmax normalization happens per node after accumulation (the den
    arrives through an all-ones table column), so no segment_max and no
    per-edge alpha materialization. Dropping the max-subtraction is safe
    here: |logits| stay O(1) for this model family.
  - The gather h_v[col] uses one indirect DMA per 128-edge chunk from a
    node table [N, 132] = [h@Wv + b | h@a_dst | ones | 0 0] rebuilt every
    layer from each core's node shard and AllGather'ed across cores.
  - Phase 0's gather of x[col] is static (x, edge_index are inputs); the
    host pre-gathers it and ships it dense.
"""

import numpy as np

import concourse.bass as bass
import concourse.mybir as mybir
import concourse.tile as tile
from concourse.bass import AP, IndirectOffsetOnAxis
from concourse.bass_utils import run_bass_kernel_spmd
from concourse.tile import ScopedClock

f32 = mybir.dt.float32
i32 = mybir.dt.int32

N = 50000
E = 800000
D_IN = 18
H = 128
HEADS = 4
DH = H // HEADS
D_OUT = 3
NEG = 0.2
BN_EPS = 1e-5

NCORES = 8
NSH = N // NCORES            # 6250 nodes per core
NBLK = (NSH + 127) // 128    # 49 blocks (48 full + one of 106)
TW = 132                     # table row width (f32)
P = 128


# ---------------------------------------------------------------------------
# container compat patches (older walrus in this image)
# ---------------------------------------------------------------------------

_patched = False


def _apply_patches():
    global _patched
    if _patched:
        return
    _patched = True

    from concourse.bass import compact_to_ranges

    # The walrus here accepts at most ONE sync-wait command per instruction,
    # and the EVSEM range-clear in the Tile tail lowers to an InstISA
    # encoding it rejects. Each kernel() call builds + loads a fresh NEFF,
    # so semaphores start zeroed and the tail clears can be dropped.
    def _drain_and_barrier(self, tick_clock, wait_clock):
        nc = self.nc
        drain_inst = nc.sync.drain()
        wait_clock.add_sem_waits(
            drain_inst.ins, ScopedClock({None: tick_clock.global_clock})
        )
        nc.all_engine_barrier()
        popped = nc._tile_sem_poison_stack.pop()
        assert popped is self._sem_poison
        sems = list(self.sems.allocated().values())
        if sems:
            sem_nums = [
                s.num if isinstance(s, bass.SemaphoreHandle) else s for s in sems
            ]
            for sem_range in compact_to_ranges(sem_nums):
                nc.gpsimd.dma_reset(sem_range)
            nc._state.prepend_free_semaphores(sem_nums)
            for poison_set in nc._tile_sem_poison_stack:
                poison_set.update(sem_nums)
        nc.all_engine_barrier()

    tile.TileContext._drain_and_barrier = _drain_and_barrier


_WAITSPLIT_CTR = [0]


def _split_multi_waits(nc, max_waits=1):
    """Move extra sync waits onto same-engine NoOps (walrus limit: 1/inst)."""
    for f in nc.m.functions:
        for b in f.blocks:
            insts = b.instructions
            i = 0
            while i < len(insts):
                inst = insts[i]
                si = inst.sync_info
                if si is not None:
                    waits = list(si.on_wait)
                    imm = [w for w in waits if w.wait_reg is None]
                    reg = [w for w in waits if w.wait_reg is not None]
                    budget = max(0, max_waits - len(reg))
                    if len(imm) > budget:
                        keep = imm[len(imm) - budget:] if budget else []
                        extras = imm[: len(imm) - budget]
                        si.on_wait = reg + keep
                        for j in range(0, len(extras), max_waits):
                            _WAITSPLIT_CTR[0] += 1
                            nop = mybir.InstNoOp(
                                name=f"I-waitsplit-{_WAITSPLIT_CTR[0]}"
                            )
                            nop.engine = inst.engine
                            nop.sync_info = mybir.SyncInfo(
                                on_wait=extras[j: j + max_waits], on_update=[]
                            )
                            insts.insert(i, nop)
                            i += 1
                i += 1


# ---------------------------------------------------------------------------
# host-side preprocessing
# ---------------------------------------------------------------------------

def _preprocess(x, edge_index):
    row = edge_index[0].astype(np.int64)
    col = edge_index[1].astype(np.int64)
    order = np.argsort(row, kind="stable")
    rs, cs = row[order], col[order]

    per_core = []
    max_chunks = np.zeros(NBLK, dtype=np.int64)
    for ci in range(NCORES):
        lo = np.searchsorted(rs, ci * NSH, "left")
        hi = np.searchsorted(rs, (ci + 1) * NSH, "left")
        r = rs[lo:hi] - ci * NSH
        c = cs[lo:hi]
        blocks = []
        for b in range(NBLK):
            blo = np.searchsorted(r, b * 128, "left")
            bhi = np.searchsorted(r, min((b + 1) * 128, NSH), "left")
            blocks.append((r[blo:bhi] - b * 128, c[blo:bhi]))
            nch = (bhi - blo + 127) // 128
            max_chunks[b] = max(max_chunks[b], nch)
        per_core.append(blocks)

    S = [max(1, int(v)) for v in max_chunks]
    C_total = int(sum(S))

    x_pad = np.zeros((N, 20), np.float32)
    x_pad[:, :D_IN] = x
    x_pad[:, D_IN] = 1.0                            # ones column -> degree

    colw = np.zeros((NCORES, P, C_total), np.int32)
    rowrel = np.full((NCORES, P, C_total), -1.0, np.float32)
    xg = np.zeros((NCORES, P, C_total, 20), np.float32)
    for ci in range(NCORES):
        k = 0
        for b in range(NBLK):
            rr, cc = per_core[ci][b]
            n = len(rr)
            for s in range(S[b]):
                a, bnd = s * 128, min((s + 1) * 128, n)
                cnt = max(0, bnd - a)
                if cnt > 0:
                    colw[ci, :cnt, k] = cc[a:bnd]
                    rowrel[ci, :cnt, k] = rr[a:bnd].astype(np.float32)
                    xg[ci, :cnt, k, :] = x_pad[cc[a:bnd]]
                k += 1
        assert k == C_total

    return S, C_total, x_pad, colw, rowrel, xg


# ---------------------------------------------------------------------------
# device kernel
# ---------------------------------------------------------------------------

def _build(S, C_total):
    nc = bass.Bass("TRN2", target_bir_lowering=False)

    d_colw = nc.dram_tensor("colw", [P, C_total], i32, kind="ExternalInput")
    d_rowrel = nc.dram_tensor("rowrel", [P, C_total], f32, kind="ExternalInput")
    d_xg = nc.dram_tensor("xg", [P, C_total * 20], f32, kind="ExternalInput")
    d_xT = nc.dram_tensor("xT", [NBLK, 20, P], f32, kind="ExternalInput")
    d_xb3 = nc.dram_tensor("xb3", [P, NBLK * 3], f32, kind="ExternalInput")
    d_wcat0 = nc.dram_tensor("wcat0", [52, P], f32, kind="ExternalInput")
    d_wv = nc.dram_tensor("wv", [3, P, TW], f32, kind="ExternalInput")
    d_wsrc1 = nc.dram_tensor("wsrc1", [P, 4], f32, kind="ExternalInput")
    d_hvb = nc.dram_tensor("hvb", [3, P], f32, kind="ExternalInput")
    d_bnsc = nc.dram_tensor("bnsc", [3, P], f32, kind="ExternalInput")
    d_bnsh = nc.dram_tensor("bnsh", [3, P], f32, kind="ExternalInput")
    d_wo1 = nc.dram_tensor("wo1", [P, P], f32, kind="ExternalInput")
    d_bo1 = nc.dram_tensor("bo1", [1, P], f32, kind="ExternalInput")
    d_wo2 = nc.dram_tensor("wo2", [P, D_OUT], f32, kind="ExternalInput")
    d_bo2 = nc.dram_tensor("bo2", [1, D_OUT], f32, kind="ExternalInput")
    d_out = nc.dram_tensor("out", [NSH, D_OUT], f32, kind="ExternalOutput")

    tloc = [nc.dram_tensor(f"tloc{l}", [NSH, TW], f32) for l in range(3)]
    tfull = [nc.dram_tensor(f"tfull{l}", [N, TW], f32, addr_space="Shared")
             for l in range(3)]
    ssrc_d = [
        nc.dram_tensor("ssrcA", [NSH, 1], f32),
        nc.dram_tensor("ssrcB", [NSH, 4], f32),
        nc.dram_tensor("ssrcC", [NSH, 1], f32),
    ]

    AL = mybir.AluOpType
    AF = mybir.ActivationFunctionType

    def blk_valid(b):
        return P if b < NBLK - 1 else NSH - (NBLK - 1) * 128

    with tile.TileContext(nc) as tc:
        with tile_pools(tc) as (res, wk, gp, up, ps, pst):

            # ---- constants / resident tiles ----
            iota_i = res.tile([P, P], i32)
            nc.gpsimd.iota(iota_i[:], pattern=[[1, P]], base=0,
                           channel_multiplier=0)
            iota_f = res.tile([P, P], f32)
            nc.vector.tensor_copy(iota_f[:], iota_i[:])
            iop_i = res.tile([P, 1], i32)
            nc.gpsimd.iota(iop_i[:], pattern=[[0, 1]], base=0,
                           channel_multiplier=1)
            iop_f = res.tile([P, 1], f32)
            nc.vector.tensor_copy(iop_f[:], iop_i[:])
            ident = res.tile([P, P], f32)
            nc.vector.tensor_scalar(out=ident[:], in0=iota_f[:],
                                    scalar1=iop_f[:], scalar2=None,
                                    op0=AL.is_equal)

            colw_t = res.tile([P, C_total], i32)
            nc.sync.dma_start(out=colw_t[:], in_=d_colw[:])
            rowrel_t = res.tile([P, C_total], f32)
            nc.sync.dma_start(out=rowrel_t[:], in_=d_rowrel[:])
            xg_t = res.tile([P, C_total * 20], f32)
            nc.sync.dma_start(out=xg_t[:], in_=d_xg[:])
            wcat0_t = res.tile([52, P], f32)
            nc.sync.dma_start(out=wcat0_t[:], in_=d_wcat0[:])
            wv_t = []
            for l in range(3):
                wvl = res.tile([P, TW], f32, tag=f"wv{l}")
                wv_t.append(wvl)
            for l in range(3):
                nc.sync.dma_start(out=wv_t[l][:], in_=d_wv[l, :, :])
            wsrc1_t = res.tile([P, 4], f32)
            nc.sync.dma_start(out=wsrc1_t[:], in_=d_wsrc1[:])
            wo1_t = res.tile([P, P], f32)
            nc.sync.dma_start(out=wo1_t[:], in_=d_wo1[:])
            wo2_t = res.tile([P, D_OUT], f32)
            nc.sync.dma_start(out=wo2_t[:], in_=d_wo2[:])
            xb3_t = res.tile([P, NBLK * 3], f32)
            nc.sync.dma_start(out=xb3_t[:], in_=d_xb3[:])

            def bcast_row(dram, off, w, tag):
                t = res.tile([P, w], f32, tag=tag)
                nc.sync.dma_start(out=t[:], in_=AP(dram, off, [[0, P], [1, w]]))
                return t

            hvb_b = [bcast_row(d_hvb, l * P, P, f"hvb{l}") for l in range(3)]
            bnsc_b = [bcast_row(d_bnsc, l * P, P, f"bnsc{l}") for l in range(3)]
            bnsh_b = [bcast_row(d_bnsh, l * P, P, f"bnsh{l}") for l in range(3)]
            bo1_b = bcast_row(d_bo1, 0, P, "bo1")
            bo2_b = bcast_row(d_bo2, 0, D_OUT, "bo2")


            def leaky_exact(dst, src):
                lp = wk.tile(list(dst.shape), f32, tag="lkp")
                nc.vector.tensor_scalar(out=lp[:], in0=src, scalar1=0.0,
                                        scalar2=None, op0=AL.max)
                ln = wk.tile(list(dst.shape), f32, tag="lkn")
                nc.vector.tensor_scalar(out=ln[:], in0=src, scalar1=0.0,
                                        scalar2=NEG, op0=AL.min, op1=AL.mult)
                nc.vector.tensor_tensor(out=dst, in0=lp[:], in1=ln[:],
                                        op=AL.add)

            def recip_newton(dst, src):
                r0 = wk.tile(list(dst.shape), f32, tag="rn0")
                nc.vector.reciprocal(r0[:], src)
                t = wk.tile(list(dst.shape), f32, tag="rnt")
                nc.vector.tensor_tensor(out=t[:], in0=src, in1=r0[:],
                                        op=AL.mult)
                nc.vector.tensor_scalar(out=t[:], in0=t[:], scalar1=-1.0,
                                        scalar2=2.0, op0=AL.mult, op1=AL.add)
                nc.vector.tensor_tensor(out=dst, in0=r0[:], in1=t[:],
                                        op=AL.mult)

            x0_res = res.tile([P, NBLK * P], f32)      # h after phase 0
            r_res = res.tile([P, NBLK * P], f32)       # h after layer 1

            # ------------- per-block: build table for layer l -------------
            def build_table(l, b, h_ap):
                v = blk_valid(b)
                tp = pst.tile([P, P], f32, space="PSUM", tag="B")
                nc.tensor.transpose(out=tp[:], in_=h_ap, identity=ident[:])
                hT = wk.tile([P, P], f32, tag="hT")
                nc.scalar.copy(hT[:], tp[:])
                tabp = ps.tile([P, TW], f32, space="PSUM", tag="A")
                nc.tensor.matmul(out=tabp[:], lhsT=hT[:], rhs=wv_t[l][:],
                                 start=True, stop=True)
                tab = wk.tile([P, TW], f32, tag="tab")
                if l == 1:
                    sp4 = ps.tile([P, TW], f32, space="PSUM", tag="A")
                    nc.tensor.matmul(out=sp4[:, 0:4], lhsT=hT[:], rhs=wsrc1_t[:],
                                     start=True, stop=True)
                    nc.scalar.copy(tab[:, 0:TW], tabp[:, 0:TW])
                    nc.vector.tensor_add(tab[:, 0:P], tab[:, 0:P], hvb_b[l][:])
                    s4 = wk.tile([P, 4], f32, tag="s4")
                    nc.scalar.copy(s4[:], sp4[:, 0:4])
                    nc.sync.dma_start(out=ssrc_d[l][b * 128: b * 128 + v, :],
                                      in_=s4[:v, :])
                else:
                    nc.scalar.copy(tab[:, 0:130], tabp[:, 0:130])
                    nc.vector.tensor_add(tab[:, 0:P], tab[:, 0:P], hvb_b[l][:])
                    nc.sync.dma_start(out=ssrc_d[l][b * 128: b * 128 + v, :],
                                      in_=tab[:v, 129:130])
                    nc.vector.memset(tab[:, 129:130], 1.0)
                    nc.vector.memset(tab[:, 130:132], 0.0)
                nc.sync.dma_start(out=tloc[l][b * 128: b * 128 + v, :],
                                  in_=tab[:v, :])

            # ------------- output head (after layer 2) -------------
            def out_head(b, h_ap):
                v = blk_valid(b)
                tp = pst.tile([P, P], f32, space="PSUM", tag="B")
                nc.tensor.transpose(out=tp[:], in_=h_ap, identity=ident[:])
                hT = wk.tile([P, P], f32, tag="hT")
                nc.scalar.copy(hT[:], tp[:])
                t1p = ps.tile([P, TW], f32, space="PSUM", tag="A")
                nc.tensor.matmul(out=t1p[:, 0:P], lhsT=hT[:], rhs=wo1_t[:],
                                 start=True, stop=True)
                t1 = wk.tile([P, P], f32, tag="t1")
                nc.vector.tensor_tensor(out=t1[:], in0=t1p[:, 0:P], in1=bo1_b[:],
                                        op=AL.add)
                leaky_exact(t1[:], t1[:])
                tp2 = pst.tile([P, P], f32, space="PSUM", tag="B")
                nc.tensor.transpose(out=tp2[:], in_=t1[:], identity=ident[:])
                t1T = wk.tile([P, P], f32, tag="t1T")
                nc.scalar.copy(t1T[:], tp2[:])
                dp = ps.tile([P, TW], f32, space="PSUM", tag="A")
                nc.tensor.matmul(out=dp[:, 0:D_OUT], lhsT=t1T[:], rhs=wo2_t[:],
                                 start=True, stop=True)
                ot = wk.tile([P, D_OUT], f32, tag="ot")
                nc.vector.tensor_tensor(out=ot[:], in0=dp[:, 0:D_OUT], in1=bo2_b[:],
                                        op=AL.add)
                nc.vector.tensor_tensor(out=ot[:], in0=ot[:],
                                        in1=xb3_t[:, b * 3:(b + 1) * 3],
                                        op=AL.add)
                nc.sync.dma_start(out=d_out[b * 128: b * 128 + v, :],
                                  in_=ot[:v, :])

            # ------------- phase 0 -------------
            k = 0
            for b in range(NBLK):
                acc = ps.tile([P, TW], f32, space="PSUM", tag="A")
                nc.vector.memset(acc[:, 0:20], 0.0)
                for s in range(S[b]):
                    U = up.tile([P, P], f32, tag="U")
                    nc.vector.tensor_scalar(out=U[:], in0=iota_f[:],
                                            scalar1=rowrel_t[:, k:k + 1],
                                            scalar2=None, op0=AL.is_equal)
                    nc.tensor.matmul(out=acc[:, 0:20], lhsT=U[:],
                                     rhs=xg_t[:, k * 20:(k + 1) * 20],
                                     start=False, stop=(s == S[b] - 1),
                                     skip_group_check=True)
                    k += 1
                den = wk.tile([P, 1], f32, tag="den")
                nc.vector.tensor_scalar(out=den[:], in0=acc[:, 18:19],
                                        scalar1=1e-8, scalar2=None, op0=AL.add)
                rec = wk.tile([P, 1], f32, tag="rec")
                recip_newton(rec[:], den[:])
                nmean52 = wk.tile([P, 52], f32, tag="nmean")
                nc.vector.tensor_scalar(out=nmean52[:, 32:50],
                                        in0=acc[:, 0:D_IN],
                                        scalar1=rec[:], scalar2=None,
                                        op0=AL.mult)
                ntp = pst.tile([P, P], f32, space="PSUM", tag="B")
                nc.tensor.transpose(out=ntp[:52, :], in_=nmean52[:],
                                    identity=ident[:])
                lhs = wk.tile([52, P], f32, tag="lhs0")
                nc.vector.memset(lhs[:], 0.0)
                nc.sync.dma_start(out=lhs[0:20, :], in_=d_xT[b, :, :])
                nc.scalar.copy(lhs[32:50, :], ntp[32:50, :])
                h0p = ps.tile([P, TW], f32, space="PSUM", tag="A")
                nc.tensor.matmul(out=h0p[:, 0:P], lhsT=lhs[:], rhs=wcat0_t[:],
                                 start=True, stop=True)
                x0_b = x0_res[:, b * P:(b + 1) * P]
                nc.scalar.copy(x0_b, h0p[:, 0:P])
                build_table(0, b, x0_b)

            def allgather(l, semname):
                tc.strict_bb_all_engine_barrier()
                with tc.tile_critical():
                    cc = nc.semaphore(semname).__enter__()
                    nc.gpsimd.collective_compute(
                        "AllGather", AL.bypass,
                        replica_groups=[list(range(NCORES))],
                        ins=[tloc[l].ap().opt()], outs=[tfull[l].ap().opt()],
                    ).then_inc(cc)
                    nc.gpsimd.wait_ge(cc, 1)
                tc.strict_bb_all_engine_barrier()

            allgather(0, "cc0")

            # ------------- attention layers -------------
            def attn_layer(l, resid_res, store_res):
                nheads = HEADS if l == 1 else 1
                k = 0
                for b in range(NBLK):
                    v = blk_valid(b)
                    nch = S[b]
                    acc = ps.tile([P, TW], f32, space="PSUM", tag="A")
                    nc.vector.memset(acc[:], 0.0)
                    gt = gp.tile([P, nch * TW], f32, tag="gt")
                    ssrcb = wk.tile([P, nheads * P], f32, tag="ssrcb")
                    nc.vector.memset(ssrcb[:], 0.0)
                    if l == 1:
                        for h in range(HEADS):
                            nc.sync.dma_start(
                                out=ssrcb[:, h * P:h * P + v],
                                in_=AP(ssrc_d[l], b * 128 * 4 + h,
                                       [[0, P], [4, v]]))
                    else:
                        nc.sync.dma_start(
                            out=ssrcb[:, 0:v],
                            in_=AP(ssrc_d[l], b * 128, [[0, P], [1, v]]))
                    scratch = wk.tile([P, P], f32, tag="scr")
                    ssrcE = wk.tile([P, nch * nheads], f32, tag="ssrcE")
                    exb = wk.tile([P, nch * nheads], f32, tag="exb")
                    k0 = k
                    for s in range(nch):
                        nc.gpsimd.indirect_dma_start(
                            out=gt[:, s * TW:(s + 1) * TW], out_offset=None,
                            in_=tfull[l][:],
                            in_offset=IndirectOffsetOnAxis(
                                ap=colw_t[:, k:k + 1], axis=0))
                        U = up.tile([P, P], f32, tag="U")
                        nc.vector.tensor_scalar(out=U[:], in0=iota_f[:],
                                                scalar1=rowrel_t[:, k:k + 1],
                                                scalar2=None, op0=AL.is_equal)
                        for h in range(nheads):
                            nc.vector.tensor_tensor(
                                out=scratch[:], in0=U[:],
                                in1=ssrcb[:, h * P:(h + 1) * P], op=AL.mult)
                            nc.vector.tensor_reduce(
                                out=ssrcE[:, s * nheads + h:s * nheads + h + 1],
                                in_=scratch[:], axis=mybir.AxisListType.X,
                                op=AL.add)
                        k += 1
                    # z / ex batched over the block's chunks
                    zt = wk.tile([P, nch * nheads], f32, tag="zt")
                    if l == 1:
                        sdst = gt[:].rearrange(
                            "p (c w) -> p c w", w=TW)[:, :, 128:132]
                        nc.vector.tensor_tensor(
                            out=zt[:].rearrange("p (c h) -> p c h", h=4),
                            in0=ssrcE[:].rearrange("p (c h) -> p c h", h=4),
                            in1=sdst, op=AL.add)
                    else:
                        sdst = gt[:].rearrange(
                            "p (c w) -> p c w", w=TW)[:, :, 128]
                        nc.vector.tensor_tensor(out=zt[:], in0=ssrcE[:],
                                                in1=sdst, op=AL.add)
                    leaky_exact(zt[:], zt[:])
                    nc.scalar.activation(out=exb[:], in_=zt[:], func=AF.Exp)
                    # weighted scatter
                    for s in range(nch):
                        kk = k0 + s
                        if l == 1:
                            rhs = up.tile([P, TW], f32, tag="rhs1")
                            for h in range(HEADS):
                                nc.vector.tensor_scalar(
                                    out=rhs[:, h * DH:(h + 1) * DH],
                                    in0=gt[:, s * TW + h * DH:
                                           s * TW + (h + 1) * DH],
                                    scalar1=exb[:, s * 4 + h:s * 4 + h + 1],
                                    scalar2=None, op0=AL.mult)
                            nc.vector.tensor_copy(
                                rhs[:, 128:132], exb[:, s * 4:s * 4 + 4])
                            U = up.tile([P, P], f32, tag="U")
                            nc.vector.tensor_scalar(
                                out=U[:], in0=iota_f[:],
                                scalar1=rowrel_t[:, kk:kk + 1],
                                scalar2=None, op0=AL.is_equal)
                            nc.tensor.matmul(
                                out=acc[:, 0:TW], lhsT=U[:], rhs=rhs[:],
                                start=False, stop=(s == nch - 1),
                                skip_group_check=True)
                        else:
                            Uw = up.tile([P, P], f32, tag="Uw")
                            nc.vector.tensor_scalar(
                                out=Uw[:], in0=iota_f[:],
                                scalar1=rowrel_t[:, kk:kk + 1],
                                scalar2=exb[:, s:s + 1],
                                op0=AL.is_equal, op1=AL.mult)
                            nc.tensor.matmul(
                                out=acc[:, 0:130], lhsT=Uw[:],
                                rhs=gt[:, s * TW:s * TW + 130],
                                start=False, stop=(s == nch - 1),
                                skip_group_check=True)
                    # ----- flush -----
                    nd = 4 if l == 1 else 1
                    den = wk.tile([P, nd], f32, tag="denL")
                    if l == 1:
                        nc.vector.tensor_scalar(out=den[:], in0=acc[:, 128:132],
                                                scalar1=1e-16, scalar2=None,
                                                op0=AL.add)
                    else:
                        nc.vector.tensor_scalar(out=den[:], in0=acc[:, 129:130],
                                                scalar1=1e-16, scalar2=None,
                                                op0=AL.add)
                    rec = wk.tile([P, nd], f32, tag="recL")
                    recip_newton(rec[:], den[:])
                    hb = wk.tile([P, P], f32, tag="hb")
                    if l == 1:
                        for h in range(HEADS):
                            nc.vector.tensor_scalar(
                                out=hb[:, h * DH:(h + 1) * DH],
                                in0=acc[:, h * DH:(h + 1) * DH],
                                scalar1=rec[:, h:h + 1], scalar2=None,
                                op0=AL.mult)
                    else:
                        nc.vector.tensor_scalar(out=hb[:], in0=acc[:, 0:P],
                                                scalar1=rec[:], scalar2=None,
                                                op0=AL.mult)
                    nc.vector.tensor_tensor(out=hb[:], in0=hb[:],
                                            in1=bnsc_b[l][:], op=AL.mult)
                    nc.vector.tensor_tensor(out=hb[:], in0=hb[:],
                                            in1=bnsh_b[l][:], op=AL.add)
                    if resid_res is not None:
                        nc.vector.tensor_tensor(
                            out=hb[:], in0=hb[:],
                            in1=resid_res[:, b * P:(b + 1) * P], op=AL.add)
                    leaky_exact(hb[:], hb[:])
                    if store_res is not None:
                        nc.vector.tensor_copy(store_res[:, b * P:(b + 1) * P],
                                              hb[:])
                        hcur = store_res[:, b * P:(b + 1) * P]
                    else:
                        hcur = hb[:]
                    if l < 2:
                        build_table(l + 1, b, hcur)
                    else:
                        out_head(b, hcur)

            attn_layer(0, x0_res, None)
            allgather(1, "cc1")
            attn_layer(1, None, r_res)
            allgather(2, "cc2")
            attn_layer(2, r_res, None)

    _split_multi_waits(nc, 1)
    return nc


class tile_pools:
    """All pools opened/closed together."""

    def __init__(self, tc):
        self.tc = tc

    def __enter__(self):
        tc = self.tc
        self.cms = [
            tc.tile_pool(name="res", bufs=1),
            tc.tile_pool(name="wk", bufs=3),
            tc.tile_pool(name="gat", bufs=4),
            tc.tile_pool(name="u", bufs=6),
            tc.tile_pool(name="ps", bufs=5, space="PSUM"),
            tc.tile_pool(name="pst", bufs=3, space="PSUM"),
        ]
        return tuple(cm.__enter__() for cm in self.cms)

    def __exit__(self, *a):
        for cm in reversed(self.cms):
            cm.__exit__(*a)
        return False


# ---------------------------------------------------------------------------
# public entry point
# ---------------------------------------------------------------------------

def prepare(inputs):
    """Build (nc, in_maps) for the SPMD run — shared with bench.py."""
    _apply_patches()
    x = np.asarray(inputs["x"], np.float32)
    edge_index = np.asarray(inputs["edge_index"], np.int32)

    S, C_total, x_pad, colw, rowrel, xg = _preprocess(x, edge_index)

    W_in = np.asarray(inputs["W_in"], np.float32)
    b_in = np.asarray(inputs["b_in"], np.float32)
    W_agg = np.asarray(inputs["W_agg"], np.float32)
    b_agg = np.asarray(inputs["b_agg"], np.float32)
    sh_Wv = np.asarray(inputs["sh_Wv"], np.float32)
    sh_b = np.asarray(inputs["sh_b"], np.float32)
    sh_asrc = np.asarray(inputs["sh_asrc"], np.float32)
    sh_adst = np.asarray(inputs["sh_adst"], np.float32)
    mh_Wv = np.asarray(inputs["mh_Wv"], np.float32)
    mh_b = np.asarray(inputs["mh_b"], np.float32)
    mh_asrc = np.asarray(inputs["mh_asrc"], np.float32)
    mh_adst = np.asarray(inputs["mh_adst"], np.float32)
    bn_g = np.asarray(inputs["bn_gamma"], np.float32)
    bn_b = np.asarray(inputs["bn_beta"], np.float32)
    bn_m = np.asarray(inputs["bn_mean"], np.float32)
    bn_v = np.asarray(inputs["bn_var"], np.float32)
    W_o1 = np.asarray(inputs["W_o1"], np.float32)
    b_o1 = np.asarray(inputs["b_o1"], np.float32)
    W_o2 = np.asarray(inputs["W_o2"], np.float32)
    b_o2 = np.asarray(inputs["b_o2"], np.float32)

    # wcat0 rows pair with lhsT rows: 0:18 x | 18 ones | 32:50 nmean
    wcat0 = np.zeros((52, P), np.float32)
    wcat0[0:18] = W_in
    wcat0[18] = b_in + b_agg
    wcat0[32:50] = W_agg

    wv = np.zeros((3, P, TW), np.float32)
    wv[0, :, 0:P] = sh_Wv[0]
    wv[0, :, 128] = sh_adst[0]
    wv[0, :, 129] = sh_asrc[0]
    wv[1, :, 0:P] = mh_Wv.transpose(1, 0, 2).reshape(P, P)
    wv[1, :, 128:132] = mh_adst.T
    wv[2, :, 0:P] = sh_Wv[1]
    wv[2, :, 128] = sh_adst[1]
    wv[2, :, 129] = sh_asrc[1]

    hvb = np.zeros((3, P), np.float32)
    hvb[0] = sh_b[0]
    hvb[1] = mh_b.reshape(P)
    hvb[2] = sh_b[1]

    bnsc = (bn_g / np.sqrt(bn_v + BN_EPS)).astype(np.float32)
    bnsh = (bn_b - bn_m * bnsc).astype(np.float32)

    xT = np.zeros((NCORES, NBLK, 20, P), np.float32)
    xb3 = np.zeros((NCORES, P, NBLK * 3), np.float32)
    for ci in range(NCORES):
        shp = np.zeros((NBLK * 128, 20), np.float32)
        shp[:NSH] = x_pad[ci * NSH:(ci + 1) * NSH]
        xT[ci] = shp.reshape(NBLK, 128, 20).transpose(0, 2, 1)
        x3 = np.zeros((NBLK * 128, 3), np.float32)
        x3[:NSH] = x[ci * NSH:(ci + 1) * NSH, -3:]
        xb3[ci] = x3.reshape(NBLK, 128, 3).transpose(1, 0, 2).reshape(
            P, NBLK * 3)

    nc = _build(S, C_total)

    in_maps = []
    for ci in range(NCORES):
        in_maps.append({
            "colw": colw[ci], "rowrel": rowrel[ci],
            "xg": xg[ci].reshape(P, C_total * 20),
            "xT": xT[ci], "xb3": xb3[ci],
            "wcat0": wcat0, "wv": wv, "wsrc1": mh_asrc.T.copy(),
            "hvb": hvb, "bnsc": bnsc, "bnsh": bnsh,
            "wo1": W_o1, "bo1": b_o1[None, :], "wo2": W_o2,
            "bo2": b_o2[None, :],
        })

    return nc, in_maps


def kernel(**inputs):
    nc, in_maps = prepare(inputs)
    res = run_bass_kernel_spmd(nc, in_maps, core_ids=list(range(NCORES)))
    out = np.concatenate([res.results[ci]["out"] for ci in range(NCORES)],
                         axis=0)
    return out.astype(np.float32)



# revision 5
# speedup vs baseline: 9.8754x; 9.8754x over previous
"""EnhancedFlowGNN forward pass on 8 Trainium2 NeuronCores (Bass/Tile).

Strategy (edge parallelism aligned with a node partition, no all-reduce):
  - Host sorts edges by destination ("row") and shards them by row range so
    core i owns nodes [i*6250, (i+1)*6250) and every edge targeting them.
  - segment_sum scatter = one-hot matmul into PSUM per 128-node block:
    U_w[e, n] = (row_rel[e] == n) * ex[e];  acc += U_w.T @ gathered_rows.
    Softmax normalization happens per node after accumulation (the den
    arrives through an all-ones table column), so no segment_max and no
    per-edge alpha materialization. Dropping the max-subtraction is safe
    here: |logits| stay O(1) for this model family.
  - The gather h_v[col] uses one indirect DMA per 128-edge chunk from a
    node table [N, 132] = [h@Wv + b | h@a_dst | ones | 0 0] rebuilt every
    layer from each core's node shard and AllGather'ed across cores.
  - Phase 0's gather of x[col] is static (x, edge_index are inputs); the
    host pre-gathers it and ships it dense.
"""

import numpy as np

import concourse.bass as bass
import concourse.mybir as mybir
import concourse.tile as tile
from concourse.bass import AP, IndirectOffsetOnAxis
from concourse.bass_utils import run_bass_kernel_spmd
from concourse.tile import ScopedClock

f32 = mybir.dt.float32
i32 = mybir.dt.int32

N = 50000
E = 800000
D_IN = 18
H = 128
HEADS = 4
DH = H // HEADS
D_OUT = 3
NEG = 0.2
BN_EPS = 1e-5

NCORES = 8
NSH = N // NCORES            # 6250 nodes per core
NBLK = (NSH + 127) // 128    # 49 blocks (48 full + one of 106)
TW = 132                     # table row width (f32)
P = 128


# ---------------------------------------------------------------------------
# container compat patches (older walrus in this image)
# ---------------------------------------------------------------------------

_patched = False


def _apply_patches():
    global _patched
    if _patched:
        return
    _patched = True

    from concourse.bass import compact_to_ranges

    # The walrus here accepts at most ONE sync-wait command per instruction,
    # and the EVSEM range-clear in the Tile tail lowers to an InstISA
    # encoding it rejects. Each kernel() call builds + loads a fresh NEFF,
    # so semaphores start zeroed and the tail clears can be dropped.
    def _drain_and_barrier(self, tick_clock, wait_clock):
        nc = self.nc
        drain_inst = nc.sync.drain()
        wait_clock.add_sem_waits(
            drain_inst.ins, ScopedClock({None: tick_clock.global_clock})
        )
        nc.all_engine_barrier()
        popped = nc._tile_sem_poison_stack.pop()
        assert popped is self._sem_poison
        sems = list(self.sems.allocated().values())
        if sems:
            sem_nums = [
                s.num if isinstance(s, bass.SemaphoreHandle) else s for s in sems
            ]
            for sem_range in compact_to_ranges(sem_nums):
                nc.gpsimd.dma_reset(sem_range)
            nc._state.prepend_free_semaphores(sem_nums)
            for poison_set in nc._tile_sem_poison_stack:
                poison_set.update(sem_nums)
        nc.all_engine_barrier()

    tile.TileContext._drain_and_barrier = _drain_and_barrier


_WAITSPLIT_CTR = [0]


def _split_multi_waits(nc, max_waits=1):
    """Move extra sync waits onto same-engine NoOps (walrus limit: 1/inst)."""
    for f in nc.m.functions:
        for b in f.blocks:
            insts = b.instructions
            i = 0
            while i < len(insts):
                inst = insts[i]
                si = inst.sync_info
                if si is not None:
                    waits = list(si.on_wait)
                    imm = [w for w in waits if w.wait_reg is None]
                    reg = [w for w in waits if w.wait_reg is not None]
                    budget = max(0, max_waits - len(reg))
                    if len(imm) > budget:
                        keep = imm[len(imm) - budget:] if budget else []
                        extras = imm[: len(imm) - budget]
                        si.on_wait = reg + keep
                        for j in range(0, len(extras), max_waits):
                            _WAITSPLIT_CTR[0] += 1
                            nop = mybir.InstNoOp(
                                name=f"I-waitsplit-{_WAITSPLIT_CTR[0]}"
                            )
                            nop.engine = inst.engine
                            nop.sync_info = mybir.SyncInfo(
                                on_wait=extras[j: j + max_waits], on_update=[]
                            )
                            insts.insert(i, nop)
                            i += 1
                i += 1


# ---------------------------------------------------------------------------
# host-side preprocessing
# ---------------------------------------------------------------------------

def _preprocess(x, edge_index):
    row = edge_index[0].astype(np.int64)
    col = edge_index[1].astype(np.int64)
    order = np.argsort(row, kind="stable")
    rs, cs = row[order], col[order]

    per_core = []
    max_chunks = np.zeros(NBLK, dtype=np.int64)
    for ci in range(NCORES):
        lo = np.searchsorted(rs, ci * NSH, "left")
        hi = np.searchsorted(rs, (ci + 1) * NSH, "left")
        r = rs[lo:hi] - ci * NSH
        c = cs[lo:hi]
        blocks = []
        for b in range(NBLK):
            blo = np.searchsorted(r, b * 128, "left")
            bhi = np.searchsorted(r, min((b + 1) * 128, NSH), "left")
            blocks.append((r[blo:bhi] - b * 128, c[blo:bhi]))
            nch = (bhi - blo + 127) // 128
            max_chunks[b] = max(max_chunks[b], nch)
        per_core.append(blocks)

    S = [max(1, int(v)) for v in max_chunks]
    C_total = int(sum(S))

    x_pad = np.zeros((N, 20), np.float32)
    x_pad[:, :D_IN] = x
    x_pad[:, D_IN] = 1.0                            # ones column -> degree

    colw = np.zeros((NCORES, P, C_total), np.int32)
    rowrel = np.full((NCORES, P, C_total), -1.0, np.float32)
    xg = np.zeros((NCORES, P, C_total, 20), np.float32)
    for ci in range(NCORES):
        k = 0
        for b in range(NBLK):
            rr, cc = per_core[ci][b]
            n = len(rr)
            for s in range(S[b]):
                a, bnd = s * 128, min((s + 1) * 128, n)
                cnt = max(0, bnd - a)
                if cnt > 0:
                    colw[ci, :cnt, k] = cc[a:bnd]
                    rowrel[ci, :cnt, k] = rr[a:bnd].astype(np.float32)
                    xg[ci, :cnt, k, :] = x_pad[cc[a:bnd]]
                k += 1
        assert k == C_total

    return S, C_total, x_pad, colw, rowrel, xg


# ---------------------------------------------------------------------------
# device kernel
# ---------------------------------------------------------------------------

def _build(S, C_total):
    nc = bass.Bass("TRN2", target_bir_lowering=False)

    d_colw = nc.dram_tensor("colw", [P, C_total], i32, kind="ExternalInput")
    d_rowrel = nc.dram_tensor("rowrel", [P, C_total], f32, kind="ExternalInput")
    d_xg = nc.dram_tensor("xg", [P, C_total * 20], f32, kind="ExternalInput")
    d_xT = nc.dram_tensor("xT", [NBLK, 20, P], f32, kind="ExternalInput")
    d_xb3 = nc.dram_tensor("xb3", [P, NBLK * 3], f32, kind="ExternalInput")
    d_wcat0 = nc.dram_tensor("wcat0", [52, P], f32, kind="ExternalInput")
    d_wv = nc.dram_tensor("wv", [3, P, TW], f32, kind="ExternalInput")
    d_wsrc1 = nc.dram_tensor("wsrc1", [P, 4], f32, kind="ExternalInput")
    d_hvb = nc.dram_tensor("hvb", [3, P], f32, kind="ExternalInput")
    d_bnsc = nc.dram_tensor("bnsc", [3, P], f32, kind="ExternalInput")
    d_bnsh = nc.dram_tensor("bnsh", [3, P], f32, kind="ExternalInput")
    d_wo1 = nc.dram_tensor("wo1", [P, P], f32, kind="ExternalInput")
    d_bo1 = nc.dram_tensor("bo1", [1, P], f32, kind="ExternalInput")
    d_wo2 = nc.dram_tensor("wo2", [P, D_OUT], f32, kind="ExternalInput")
    d_bo2 = nc.dram_tensor("bo2", [1, D_OUT], f32, kind="ExternalInput")
    d_out = nc.dram_tensor("out", [NSH, D_OUT], f32, kind="ExternalOutput")

    tloc = [nc.dram_tensor(f"tloc{l}", [NSH, TW], f32) for l in range(3)]
    tfull = [nc.dram_tensor(f"tfull{l}", [N, TW], f32, addr_space="Shared")
             for l in range(3)]
    ssrc_d = [
        nc.dram_tensor("ssrcA", [NSH, 1], f32),
        nc.dram_tensor("ssrcB", [NSH, 4], f32),
        nc.dram_tensor("ssrcC", [NSH, 1], f32),
    ]

    AL = mybir.AluOpType
    AF = mybir.ActivationFunctionType

    def blk_valid(b):
        return P if b < NBLK - 1 else NSH - (NBLK - 1) * 128

    with tile.TileContext(nc) as tc:
        with tile_pools(tc) as (res, wk, gp, up, ps, pst):

            # ---- constants / resident tiles ----
            iota_i = res.tile([P, P], i32)
            nc.gpsimd.iota(iota_i[:], pattern=[[1, P]], base=0,
                           channel_multiplier=0)
            iota_f = res.tile([P, P], f32)
            nc.vector.tensor_copy(iota_f[:], iota_i[:])
            iop_i = res.tile([P, 1], i32)
            nc.gpsimd.iota(iop_i[:], pattern=[[0, 1]], base=0,
                           channel_multiplier=1)
            iop_f = res.tile([P, 1], f32)
            nc.vector.tensor_copy(iop_f[:], iop_i[:])
            ident = res.tile([P, P], f32)
            nc.vector.tensor_scalar(out=ident[:], in0=iota_f[:],
                                    scalar1=iop_f[:], scalar2=None,
                                    op0=AL.is_equal)

            colw_t = res.tile([P, C_total], i32)
            nc.sync.dma_start(out=colw_t[:], in_=d_colw[:])
            rowrel_t = res.tile([P, C_total], f32)
            nc.sync.dma_start(out=rowrel_t[:], in_=d_rowrel[:])
            xg_t = res.tile([P, C_total * 20], f32)
            nc.sync.dma_start(out=xg_t[:], in_=d_xg[:])
            wcat0_t = res.tile([52, P], f32)
            nc.sync.dma_start(out=wcat0_t[:], in_=d_wcat0[:])
            wv_t = []
            for l in range(3):
                wvl = res.tile([P, TW], f32, tag=f"wv{l}")
                wv_t.append(wvl)
            for l in range(3):
                nc.sync.dma_start(out=wv_t[l][:], in_=d_wv[l, :, :])
            wsrc1_t = res.tile([P, 4], f32)
            nc.sync.dma_start(out=wsrc1_t[:], in_=d_wsrc1[:])
            wo1_t = res.tile([P, P], f32)
            nc.sync.dma_start(out=wo1_t[:], in_=d_wo1[:])
            wo2_t = res.tile([P, D_OUT], f32)
            nc.sync.dma_start(out=wo2_t[:], in_=d_wo2[:])
            xb3_t = res.tile([P, NBLK * 3], f32)
            nc.sync.dma_start(out=xb3_t[:], in_=d_xb3[:])

            def bcast_row(dram, off, w, tag):
                t = res.tile([P, w], f32, tag=tag)
                nc.sync.dma_start(out=t[:], in_=AP(dram, off, [[0, P], [1, w]]))
                return t

            hvb_b = [bcast_row(d_hvb, l * P, P, f"hvb{l}") for l in range(3)]
            bnsc_b = [bcast_row(d_bnsc, l * P, P, f"bnsc{l}") for l in range(3)]
            bnsh_b = [bcast_row(d_bnsh, l * P, P, f"bnsh{l}") for l in range(3)]
            bo1_b = bcast_row(d_bo1, 0, P, "bo1")
            bo2_b = bcast_row(d_bo2, 0, D_OUT, "bo2")


            def leaky_exact(dst, src):
                lp = wk.tile(list(dst.shape), f32, tag="lkp")
                nc.vector.tensor_scalar(out=lp[:], in0=src, scalar1=0.0,
                                        scalar2=None, op0=AL.max)
                ln = wk.tile(list(dst.shape), f32, tag="lkn")
                nc.vector.tensor_scalar(out=ln[:], in0=src, scalar1=0.0,
                                        scalar2=NEG, op0=AL.min, op1=AL.mult)
                nc.vector.tensor_tensor(out=dst, in0=lp[:], in1=ln[:],
                                        op=AL.add)

            def recip_newton(dst, src):
                r0 = wk.tile(list(dst.shape), f32, tag="rn0")
                nc.vector.reciprocal(r0[:], src)
                t = wk.tile(list(dst.shape), f32, tag="rnt")
                nc.vector.tensor_tensor(out=t[:], in0=src, in1=r0[:],
                                        op=AL.mult)
                nc.vector.tensor_scalar(out=t[:], in0=t[:], scalar1=-1.0,
                                        scalar2=2.0, op0=AL.mult, op1=AL.add)
                nc.vector.tensor_tensor(out=dst, in0=r0[:], in1=t[:],
                                        op=AL.mult)

            x0_res = res.tile([P, NBLK * P], f32)      # h after phase 0
            r_res = res.tile([P, NBLK * P], f32)       # h after layer 1

            # ------------- per-block: build table for layer l -------------
            def build_table(l, b, h_ap):
                v = blk_valid(b)
                tp = pst.tile([P, P], f32, space="PSUM", tag="B")
                nc.tensor.transpose(out=tp[:], in_=h_ap, identity=ident[:])
                hT = wk.tile([P, P], f32, tag="hT")
                nc.scalar.copy(hT[:], tp[:])
                tabp = ps.tile([P, TW], f32, space="PSUM", tag="A")
                nc.tensor.matmul(out=tabp[:], lhsT=hT[:], rhs=wv_t[l][:],
                                 start=True, stop=True)
                tab = wk.tile([P, TW], f32, tag="tab")
                if l == 1:
                    sp4 = ps.tile([P, TW], f32, space="PSUM", tag="A")
                    nc.tensor.matmul(out=sp4[:, 0:4], lhsT=hT[:], rhs=wsrc1_t[:],
                                     start=True, stop=True)
                    nc.scalar.copy(tab[:, 0:TW], tabp[:, 0:TW])
                    nc.vector.tensor_add(tab[:, 0:P], tab[:, 0:P], hvb_b[l][:])
                    s4 = wk.tile([P, 4], f32, tag="s4")
                    nc.scalar.copy(s4[:], sp4[:, 0:4])
                    nc.sync.dma_start(out=ssrc_d[l][b * 128: b * 128 + v, :],
                                      in_=s4[:v, :])
                else:
                    nc.scalar.copy(tab[:, 0:130], tabp[:, 0:130])
                    nc.vector.tensor_add(tab[:, 0:P], tab[:, 0:P], hvb_b[l][:])
                    nc.sync.dma_start(out=ssrc_d[l][b * 128: b * 128 + v, :],
                                      in_=tab[:v, 129:130])
                    nc.vector.memset(tab[:, 129:130], 1.0)
                    nc.vector.memset(tab[:, 130:132], 0.0)
                nc.sync.dma_start(out=tloc[l][b * 128: b * 128 + v, :],
                                  in_=tab[:v, :])

            # ------------- output head (after layer 2) -------------
            def out_head(b, h_ap):
                v = blk_valid(b)
                tp = pst.tile([P, P], f32, space="PSUM", tag="B")
                nc.tensor.transpose(out=tp[:], in_=h_ap, identity=ident[:])
                hT = wk.tile([P, P], f32, tag="hT")
                nc.scalar.copy(hT[:], tp[:])
                t1p = ps.tile([P, TW], f32, space="PSUM", tag="A")
                nc.tensor.matmul(out=t1p[:, 0:P], lhsT=hT[:], rhs=wo1_t[:],
                                 start=True, stop=True)
                t1 = wk.tile([P, P], f32, tag="t1")
                nc.vector.tensor_tensor(out=t1[:], in0=t1p[:, 0:P], in1=bo1_b[:],
                                        op=AL.add)
                leaky_exact(t1[:], t1[:])
                tp2 = pst.tile([P, P], f32, space="PSUM", tag="B")
                nc.tensor.transpose(out=tp2[:], in_=t1[:], identity=ident[:])
                t1T = wk.tile([P, P], f32, tag="t1T")
                nc.scalar.copy(t1T[:], tp2[:])
                dp = ps.tile([P, TW], f32, space="PSUM", tag="A")
                nc.tensor.matmul(out=dp[:, 0:D_OUT], lhsT=t1T[:], rhs=wo2_t[:],
                                 start=True, stop=True)
                ot = wk.tile([P, D_OUT], f32, tag="ot")
                nc.vector.tensor_tensor(out=ot[:], in0=dp[:, 0:D_OUT], in1=bo2_b[:],
                                        op=AL.add)
                nc.vector.tensor_tensor(out=ot[:], in0=ot[:],
                                        in1=xb3_t[:, b * 3:(b + 1) * 3],
                                        op=AL.add)
                nc.sync.dma_start(out=d_out[b * 128: b * 128 + v, :],
                                  in_=ot[:v, :])

            # ------------- phase 0 -------------
            k = 0
            for b in range(NBLK):
                acc = ps.tile([P, TW], f32, space="PSUM", tag="A")
                nc.vector.memset(acc[:, 0:20], 0.0)
                for s in range(S[b]):
                    U = up.tile([P, P], f32, tag="U")
                    nc.vector.tensor_scalar(out=U[:], in0=iota_f[:],
                                            scalar1=rowrel_t[:, k:k + 1],
                                            scalar2=None, op0=AL.is_equal)
                    nc.tensor.matmul(out=acc[:, 0:20], lhsT=U[:],
                                     rhs=xg_t[:, k * 20:(k + 1) * 20],
                                     start=False, stop=(s == S[b] - 1),
                                     skip_group_check=True)
                    k += 1
                den = wk.tile([P, 1], f32, tag="den")
                nc.vector.tensor_scalar(out=den[:], in0=acc[:, 18:19],
                                        scalar1=1e-8, scalar2=None, op0=AL.add)
                rec = wk.tile([P, 1], f32, tag="rec")
                recip_newton(rec[:], den[:])
                nmean52 = wk.tile([P, 52], f32, tag="nmean")
                nc.vector.tensor_scalar(out=nmean52[:, 32:50],
                                        in0=acc[:, 0:D_IN],
                                        scalar1=rec[:], scalar2=None,
                                        op0=AL.mult)
                ntp = pst.tile([P, P], f32, space="PSUM", tag="B")
                nc.tensor.transpose(out=ntp[:52, :], in_=nmean52[:],
                                    identity=ident[:])
                lhs = wk.tile([52, P], f32, tag="lhs0")
                nc.vector.memset(lhs[:], 0.0)
                nc.sync.dma_start(out=lhs[0:20, :], in_=d_xT[b, :, :])
                nc.scalar.copy(lhs[32:50, :], ntp[32:50, :])
                h0p = ps.tile([P, TW], f32, space="PSUM", tag="A")
                nc.tensor.matmul(out=h0p[:, 0:P], lhsT=lhs[:], rhs=wcat0_t[:],
                                 start=True, stop=True)
                x0_b = x0_res[:, b * P:(b + 1) * P]
                nc.scalar.copy(x0_b, h0p[:, 0:P])
                build_table(0, b, x0_b)

            def allgather(l, semname):
                tc.strict_bb_all_engine_barrier()
                with tc.tile_critical():
                    cc = nc.semaphore(semname).__enter__()
                    nc.gpsimd.collective_compute(
                        "AllGather", AL.bypass,
                        replica_groups=[list(range(NCORES))],
                        ins=[tloc[l].ap().opt()], outs=[tfull[l].ap().opt()],
                    ).then_inc(cc)
                    nc.gpsimd.wait_ge(cc, 1)
                tc.strict_bb_all_engine_barrier()

            import os
            if os.environ.get("BASS_GNN_STAGE", "full") != "p0":
                allgather(0, "cc0")

            # ------------- attention layers -------------
            def attn_layer(l, resid_res, store_res):
                nheads = HEADS if l == 1 else 1
                k = 0
                for b in range(NBLK):
                    v = blk_valid(b)
                    nch = S[b]
                    acc = ps.tile([P, TW], f32, space="PSUM", tag="A")
                    nc.vector.memset(acc[:], 0.0)
                    gt = gp.tile([P, nch * TW], f32, tag="gt")
                    ssrcb = wk.tile([P, nheads * P], f32, tag="ssrcb")
                    nc.vector.memset(ssrcb[:], 0.0)
                    if l == 1:
                        for h in range(HEADS):
                            nc.sync.dma_start(
                                out=ssrcb[:, h * P:h * P + v],
                                in_=AP(ssrc_d[l], b * 128 * 4 + h,
                                       [[0, P], [4, v]]))
                    else:
                        nc.sync.dma_start(
                            out=ssrcb[:, 0:v],
                            in_=AP(ssrc_d[l], b * 128, [[0, P], [1, v]]))
                    scratch = wk.tile([P, P], f32, tag="scr")
                    ssrcE = wk.tile([P, nch * nheads], f32, tag="ssrcE")
                    exb = wk.tile([P, nch * nheads], f32, tag="exb")
                    k0 = k
                    for s in range(nch):
                        nc.gpsimd.indirect_dma_start(
                            out=gt[:, s * TW:(s + 1) * TW], out_offset=None,
                            in_=tfull[l][:],
                            in_offset=IndirectOffsetOnAxis(
                                ap=colw_t[:, k:k + 1], axis=0))
                        U = up.tile([P, P], f32, tag="U")
                        nc.vector.tensor_scalar(out=U[:], in0=iota_f[:],
                                                scalar1=rowrel_t[:, k:k + 1],
                                                scalar2=None, op0=AL.is_equal)
                        for h in range(nheads):
                            nc.vector.tensor_tensor(
                                out=scratch[:], in0=U[:],
                                in1=ssrcb[:, h * P:(h + 1) * P], op=AL.mult)
                            nc.vector.tensor_reduce(
                                out=ssrcE[:, s * nheads + h:s * nheads + h + 1],
                                in_=scratch[:], axis=mybir.AxisListType.X,
                                op=AL.add)
                        k += 1
                    # z / ex batched over the block's chunks
                    zt = wk.tile([P, nch * nheads], f32, tag="zt")
                    if l == 1:
                        sdst = gt[:].rearrange(
                            "p (c w) -> p c w", w=TW)[:, :, 128:132]
                        nc.vector.tensor_tensor(
                            out=zt[:].rearrange("p (c h) -> p c h", h=4),
                            in0=ssrcE[:].rearrange("p (c h) -> p c h", h=4),
                            in1=sdst, op=AL.add)
                    else:
                        sdst = gt[:].rearrange(
                            "p (c w) -> p c w", w=TW)[:, :, 128]
                        nc.vector.tensor_tensor(out=zt[:], in0=ssrcE[:],
                                                in1=sdst, op=AL.add)
                    leaky_exact(zt[:], zt[:])
                    nc.scalar.activation(out=exb[:], in_=zt[:], func=AF.Exp)
                    # weighted scatter
                    for s in range(nch):
                        kk = k0 + s
                        if l == 1:
                            rhs = up.tile([P, TW], f32, tag="rhs1")
                            for h in range(HEADS):
                                nc.vector.tensor_scalar(
                                    out=rhs[:, h * DH:(h + 1) * DH],
                                    in0=gt[:, s * TW + h * DH:
                                           s * TW + (h + 1) * DH],
                                    scalar1=exb[:, s * 4 + h:s * 4 + h + 1],
                                    scalar2=None, op0=AL.mult)
                            nc.vector.tensor_copy(
                                rhs[:, 128:132], exb[:, s * 4:s * 4 + 4])
                            U = up.tile([P, P], f32, tag="U")
                            nc.vector.tensor_scalar(
                                out=U[:], in0=iota_f[:],
                                scalar1=rowrel_t[:, kk:kk + 1],
                                scalar2=None, op0=AL.is_equal)
                            nc.tensor.matmul(
                                out=acc[:, 0:TW], lhsT=U[:], rhs=rhs[:],
                                start=False, stop=(s == nch - 1),
                                skip_group_check=True)
                        else:
                            Uw = up.tile([P, P], f32, tag="Uw")
                            nc.vector.tensor_scalar(
                                out=Uw[:], in0=iota_f[:],
                                scalar1=rowrel_t[:, kk:kk + 1],
                                scalar2=exb[:, s:s + 1],
                                op0=AL.is_equal, op1=AL.mult)
                            nc.tensor.matmul(
                                out=acc[:, 0:130], lhsT=Uw[:],
                                rhs=gt[:, s * TW:s * TW + 130],
                                start=False, stop=(s == nch - 1),
                                skip_group_check=True)
                    # ----- flush -----
                    nd = 4 if l == 1 else 1
                    den = wk.tile([P, nd], f32, tag="denL")
                    if l == 1:
                        nc.vector.tensor_scalar(out=den[:], in0=acc[:, 128:132],
                                                scalar1=1e-16, scalar2=None,
                                                op0=AL.add)
                    else:
                        nc.vector.tensor_scalar(out=den[:], in0=acc[:, 129:130],
                                                scalar1=1e-16, scalar2=None,
                                                op0=AL.add)
                    rec = wk.tile([P, nd], f32, tag="recL")
                    recip_newton(rec[:], den[:])
                    hb = wk.tile([P, P], f32, tag="hb")
                    if l == 1:
                        for h in range(HEADS):
                            nc.vector.tensor_scalar(
                                out=hb[:, h * DH:(h + 1) * DH],
                                in0=acc[:, h * DH:(h + 1) * DH],
                                scalar1=rec[:, h:h + 1], scalar2=None,
                                op0=AL.mult)
                    else:
                        nc.vector.tensor_scalar(out=hb[:], in0=acc[:, 0:P],
                                                scalar1=rec[:], scalar2=None,
                                                op0=AL.mult)
                    nc.vector.tensor_tensor(out=hb[:], in0=hb[:],
                                            in1=bnsc_b[l][:], op=AL.mult)
                    nc.vector.tensor_tensor(out=hb[:], in0=hb[:],
                                            in1=bnsh_b[l][:], op=AL.add)
                    if resid_res is not None:
                        nc.vector.tensor_tensor(
                            out=hb[:], in0=hb[:],
                            in1=resid_res[:, b * P:(b + 1) * P], op=AL.add)
                    leaky_exact(hb[:], hb[:])
                    if store_res is not None:
                        nc.vector.tensor_copy(store_res[:, b * P:(b + 1) * P],
                                              hb[:])
                        hcur = store_res[:, b * P:(b + 1) * P]
                    else:
                        hcur = hb[:]
                    if l < 2:
                        build_table(l + 1, b, hcur)
                    else:
                        out_head(b, hcur)

            import os
            stage = os.environ.get("BASS_GNN_STAGE", "full")
            if stage != "p0":
                attn_layer(0, x0_res, None)
                if stage not in ("l0",):
                    allgather(1, "cc1")
                    attn_layer(1, None, r_res)
                    if stage not in ("l01",):
                        allgather(2, "cc2")
                        attn_layer(2, r_res, None)

    _split_multi_waits(nc, 1)
    return nc


class tile_pools:
    """All pools opened/closed together."""

    def __init__(self, tc):
        self.tc = tc

    def __enter__(self):
        tc = self.tc
        self.cms = [
            tc.tile_pool(name="res", bufs=1),
            tc.tile_pool(name="wk", bufs=3),
            tc.tile_pool(name="gat", bufs=4),
            tc.tile_pool(name="u", bufs=6),
            tc.tile_pool(name="ps", bufs=5, space="PSUM"),
            tc.tile_pool(name="pst", bufs=3, space="PSUM"),
        ]
        return tuple(cm.__enter__() for cm in self.cms)

    def __exit__(self, *a):
        for cm in reversed(self.cms):
            cm.__exit__(*a)
        return False


# ---------------------------------------------------------------------------
# public entry point
# ---------------------------------------------------------------------------

def prepare(inputs):
    """Build (nc, in_maps) for the SPMD run — shared with bench.py."""
    _apply_patches()
    x = np.asarray(inputs["x"], np.float32)
    edge_index = np.asarray(inputs["edge_index"], np.int32)

    S, C_total, x_pad, colw, rowrel, xg = _preprocess(x, edge_index)

    W_in = np.asarray(inputs["W_in"], np.float32)
    b_in = np.asarray(inputs["b_in"], np.float32)
    W_agg = np.asarray(inputs["W_agg"], np.float32)
    b_agg = np.asarray(inputs["b_agg"], np.float32)
    sh_Wv = np.asarray(inputs["sh_Wv"], np.float32)
    sh_b = np.asarray(inputs["sh_b"], np.float32)
    sh_asrc = np.asarray(inputs["sh_asrc"], np.float32)
    sh_adst = np.asarray(inputs["sh_adst"], np.float32)
    mh_Wv = np.asarray(inputs["mh_Wv"], np.float32)
    mh_b = np.asarray(inputs["mh_b"], np.float32)
    mh_asrc = np.asarray(inputs["mh_asrc"], np.float32)
    mh_adst = np.asarray(inputs["mh_adst"], np.float32)
    bn_g = np.asarray(inputs["bn_gamma"], np.float32)
    bn_b = np.asarray(inputs["bn_beta"], np.float32)
    bn_m = np.asarray(inputs["bn_mean"], np.float32)
    bn_v = np.asarray(inputs["bn_var"], np.float32)
    W_o1 = np.asarray(inputs["W_o1"], np.float32)
    b_o1 = np.asarray(inputs["b_o1"], np.float32)
    W_o2 = np.asarray(inputs["W_o2"], np.float32)
    b_o2 = np.asarray(inputs["b_o2"], np.float32)

    # wcat0 rows pair with lhsT rows: 0:18 x | 18 ones | 32:50 nmean
    wcat0 = np.zeros((52, P), np.float32)
    wcat0[0:18] = W_in
    wcat0[18] = b_in + b_agg
    wcat0[32:50] = W_agg

    wv = np.zeros((3, P, TW), np.float32)
    wv[0, :, 0:P] = sh_Wv[0]
    wv[0, :, 128] = sh_adst[0]
    wv[0, :, 129] = sh_asrc[0]
    wv[1, :, 0:P] = mh_Wv.transpose(1, 0, 2).reshape(P, P)
    wv[1, :, 128:132] = mh_adst.T
    wv[2, :, 0:P] = sh_Wv[1]
    wv[2, :, 128] = sh_adst[1]
    wv[2, :, 129] = sh_asrc[1]

    hvb = np.zeros((3, P), np.float32)
    hvb[0] = sh_b[0]
    hvb[1] = mh_b.reshape(P)
    hvb[2] = sh_b[1]

    bnsc = (bn_g / np.sqrt(bn_v + BN_EPS)).astype(np.float32)
    bnsh = (bn_b - bn_m * bnsc).astype(np.float32)

    xT = np.zeros((NCORES, NBLK, 20, P), np.float32)
    xb3 = np.zeros((NCORES, P, NBLK * 3), np.float32)
    for ci in range(NCORES):
        shp = np.zeros((NBLK * 128, 20), np.float32)
        shp[:NSH] = x_pad[ci * NSH:(ci + 1) * NSH]
        xT[ci] = shp.reshape(NBLK, 128, 20).transpose(0, 2, 1)
        x3 = np.zeros((NBLK * 128, 3), np.float32)
        x3[:NSH] = x[ci * NSH:(ci + 1) * NSH, -3:]
        xb3[ci] = x3.reshape(NBLK, 128, 3).transpose(1, 0, 2).reshape(
            P, NBLK * 3)

    nc = _build(S, C_total)

    in_maps = []
    for ci in range(NCORES):
        in_maps.append({
            "colw": colw[ci], "rowrel": rowrel[ci],
            "xg": xg[ci].reshape(P, C_total * 20),
            "xT": xT[ci], "xb3": xb3[ci],
            "wcat0": wcat0, "wv": wv, "wsrc1": mh_asrc.T.copy(),
            "hvb": hvb, "bnsc": bnsc, "bnsh": bnsh,
            "wo1": W_o1, "bo1": b_o1[None, :], "wo2": W_o2,
            "bo2": b_o2[None, :],
        })

    return nc, in_maps


def kernel(**inputs):
    nc, in_maps = prepare(inputs)
    res = run_bass_kernel_spmd(nc, in_maps, core_ids=list(range(NCORES)))
    out = np.concatenate([res.results[ci]["out"] for ci in range(NCORES)],
                         axis=0)
    return out.astype(np.float32)

